# revision 9
# baseline (speedup 1.0000x reference)
"""Trainium2 Bass kernel for masked GNN message passing (AdjacencyControl).

Computes, for fixed shapes N=100000 nodes, E edges, D=128 features:
    h   = x @ W.T + b
    out[i] = sum over edges (i, j) of (node_rankings[j] <= 10000) * h[j]

Strategy (8 NeuronCores, SPMD, no collectives):
  host: integer-only preprocessing — drop edges whose source is masked
        out, compact masked source nodes into a dense table, sort kept
        edges by destination, shard edges by destination range
        (N/8 nodes per core), pad each 128-row destination block to a
        fixed number of 128-edge chunks.
  core: (A) h_masked = x_masked @ W.T + b via TensorE, streamed to a
        DRAM table; (B) dma_gather pulls the per-edge source rows of
        the table into SBUF; (C) scatter-add realised as one-hot
        matmuls accumulated in PSUM per 128-row output block.
"""

import math
import os
import sys

import numpy as np

for _p in ("/opt/trn_rl_repo", "/root/.axon_site/_ro/trn_rl_repo"):
    if os.path.isdir(_p) and _p not in sys.path:
        sys.path.append(_p)

import concourse.bass as bass
import concourse.mybir as mybir
import concourse.tile as tile
from concourse import bacc
from concourse.bass import ts
from concourse.bass_utils import run_bass_kernel_spmd

P = 128          # partitions / tile edge
D = 128          # feature dim
M = 8            # cores
K_RANK = 10000   # ranking threshold from the reference model

_cache: dict = {}
TRACE = False      # set True to capture an NTFF profile (slower dispatch)
LAST = {}          # exec_time_ns / profile info from the last run


def _preprocess(x, W, b, edge_index, node_rankings):
    N = x.shape[0]
    nsh = -(-N // M)                    # nodes per core shard
    nsh_pad = -(-nsh // P) * P
    nblocks = nsh_pad // P

    mask = node_rankings <= K_RANK
    row = edge_index[0].astype(np.int64)
    col = edge_index[1].astype(np.int64)
    keep = mask[col]
    row = row[keep]
    col = col[keep]

    masked_nodes = np.flatnonzero(mask)
    nm = len(masked_nodes)
    nm_pad = max(P, -(-nm // P) * P)
    assert nm_pad <= 32512, (
        f"{nm} masked nodes exceeds the int16 gather-index capacity; "
        "this build only supports <=32512 masked source nodes"
    )
    remap = np.zeros(N, np.int64)
    remap[masked_nodes] = np.arange(nm)
    srcc = remap[col]

    order = np.argsort(row, kind="stable")
    row = row[order]
    srcc = srcc[order]

    core_of = row // nsh
    dst_local = row - core_of * nsh
    blk = dst_local // P
    gb = core_of * nblocks + blk                       # global block id
    counts = np.bincount(gb, minlength=M * nblocks)
    kc = max(2, -(-int(counts.max()) // P)) if len(row) else 2
    cap = kc * P

    group_start = np.zeros(M * nblocks, np.int64)
    np.cumsum(counts[:-1], out=group_start[1:])
    rank = np.arange(len(row)) - group_start[gb]
    slot = gb * cap + rank

    src_pad = np.zeros(M * nblocks * cap, np.int16)
    dstr_pad = np.full(M * nblocks * cap, -1.0, np.float32)
    src_pad[slot] = srcc.astype(np.int16)
    dstr_pad[slot] = (dst_local - blk * P).astype(np.float32)

    npad = nblocks * cap                               # padded edges per core
    nchunks = npad // P                                # = nblocks * kc

    # dma_gather index layout: index i lives at [partition i%16, free i//16],
    # replicated to all 8 groups of 16 partitions.
    gidx = src_pad.reshape(M, npad // 16, 16).transpose(0, 2, 1)
    gidx = np.ascontiguousarray(np.tile(gidx, (1, 8, 1)))

    # per-chunk destination offsets, partition-major: [M, 128, nchunks]
    dstr = np.ascontiguousarray(
        dstr_pad.reshape(M, nchunks, P).transpose(0, 2, 1)
    )

    xmt = np.zeros((D, nm_pad), np.float32)
    xmt[:, :nm] = x[masked_nodes].T
    wt = np.ascontiguousarray(W.T.astype(np.float32))
    bias = np.tile(b.astype(np.float32)[None, :], (P, 1))
    iota = np.tile(np.arange(P, dtype=np.float32)[None, :], (P, 1))

    meta = dict(
        N=N, nsh=nsh, nsh_pad=nsh_pad, nblocks=nblocks,
        nm_pad=nm_pad, kc=kc, nchunks=nchunks, npad=npad,
    )
    per_core = [
        {
            "xmt": xmt, "wt": wt, "bias": bias, "iota": iota,
            "gidx": np.ascontiguousarray(gidx[i]),
            "dstr": dstr[i],
        }
        for i in range(M)
    ]
    return meta, per_core


def _build(meta, stage=3):
    nm_pad = meta["nm_pad"]
    nsh_pad = meta["nsh_pad"]
    nblocks = meta["nblocks"]
    kc = meta["kc"]
    nchunks = meta["nchunks"]
    npad = meta["npad"]
    nt_h = nm_pad // P

    # SWDGE descriptor-ring limit: at most 1024 gather indices per
    # dma_gather instruction (HW-verified; 1280+ wedges the device).
    GC = 8                                             # chunks per gather
    ngather = -(-nchunks // GC)
    OB = 16                                            # out blocks per DMA

    f32 = mybir.dt.float32
    nc = bacc.Bacc("TRN2", target_bir_lowering=False, debug=False,
                   num_devices=M, num_swdge_queues=4)

    xmt_d = nc.declare_dram_parameter("xmt", [D, nm_pad], f32, isOutput=False)
    wt_d = nc.declare_dram_parameter("wt", [D, D], f32, isOutput=False)
    bias_d = nc.declare_dram_parameter("bias", [P, D], f32, isOutput=False)
    iota_d = nc.declare_dram_parameter("iota", [P, P], f32, isOutput=False)
    gidx_d = nc.declare_dram_parameter(
        "gidx", [P, npad // 16], mybir.dt.int16, isOutput=False)
    dstr_d = nc.declare_dram_parameter(
        "dstr", [P, nchunks], f32, isOutput=False)
    out_d = nc.declare_dram_parameter(
        "out", [nsh_pad, D], f32, isOutput=True)
    hm_d = nc.dram_tensor("hm", [nm_pad, D], f32)

    HB = 4                                             # h tiles per store DMA

    with tile.TileContext(nc) as tc:
        with (
            tc.tile_pool(name="consts", bufs=1) as cpool,
            tc.tile_pool(name="xmt", bufs=1) as xpool,
            tc.tile_pool(name="hstage", bufs=3) as hpool,
            tc.tile_pool(name="msg", bufs=4) as mpool,
            tc.tile_pool(name="ptile", bufs=6) as ppool,
            tc.tile_pool(name="ostage", bufs=2) as opool,
            tc.tile_pool(name="psum_h", bufs=4, space="PSUM") as psh,
            tc.tile_pool(name="psum_o", bufs=4, space="PSUM") as pso,
        ):
            wt_t = cpool.tile([D, D], f32)
            nc.sync.dma_start(out=wt_t[:], in_=wt_d.ap())
            bias_t = cpool.tile([P, D], f32)
            nc.sync.dma_start(out=bias_t[:], in_=bias_d.ap())
            iota_t = cpool.tile([P, P], f32)
            nc.sync.dma_start(out=iota_t[:], in_=iota_d.ap())
            gidx_t = cpool.tile([P, npad // 16], mybir.dt.int16)
            nc.sync.dma_start(out=gidx_t[:], in_=gidx_d.ap())
            dstr_t = cpool.tile([P, nchunks], f32)
            nc.sync.dma_start(out=dstr_t[:], in_=dstr_d.ap())

            xmt_t = xpool.tile([D, nm_pad], f32)
            nslice = 4
            step = -(-nt_h // nslice) * P
            for s in range(0, nm_pad, step):
                e = min(s + step, nm_pad)
                nc.sync.dma_start(out=xmt_t[:, s:e], in_=xmt_d.ap()[:, s:e])

            # Phase A: h = x_masked @ W.T + b -> DRAM table, node-major
            hm_r = hm_d.ap().rearrange("(t p) d -> p t d", p=P)
            for t0 in range(0, nt_h, HB):
                t1 = min(t0 + HB, nt_h)
                hs = hpool.tile([P, HB, D], f32, tag="hs")
                for t in range(t0, t1):
                    ph = psh.tile([P, D], f32)
                    nc.tensor.matmul(out=ph[:], lhsT=xmt_t[:, ts(t, P)],
                                     rhs=wt_t[:], start=True, stop=True)
                    nc.vector.tensor_add(out=hs[:, t - t0, :], in0=ph[:],
                                         in1=bias_t[:])
                nc.sync.dma_start(out=hm_r[:, t0:t1, :],
                                  in_=hs[:, : t1 - t0, :])

            # Phases B+C: gather message rows, one-hot matmul scatter-add
            out_r = out_d.ap().rearrange("(t p) d -> p t d", p=P)
            mbs = {}           # gather group -> (msg tile, chunk offset)
            ost = None
            o0 = 0             # first block staged in ost
            po = None
            for blk in range(nblocks):
                for j in range(kc):
                    c = blk * kc + j
                    gi = c // GC
                    if gi not in mbs:
                        c0 = gi * GC
                        nch = min(GC, nchunks - c0)
                        mb = mpool.tile([P, GC, D], f32, tag="mb")
                        if stage >= 2:
                            nc.gpsimd.dma_gather(
                                out_ap=mb[:, :nch, :],
                                in_ap=hm_d.ap(),
                                idxs_ap=gidx_t[:, c0 * 8:(c0 + nch) * 8],
                                num_idxs=nch * P,
                                num_idxs_reg=nch * P,
                                elem_size=D,
                                queue_num=gi % 4,
                            )
                        else:
                            nc.vector.memset(mb[:], 0.0)
                        mbs = {gi: (mb, c0)}
                    mb, c0 = mbs[gi]
                    if stage < 3:
                        continue
                    if j == 0:
                        po = pso.tile([P, D], f32)
                    pt = ppool.tile([P, P], f32, tag="pt")
                    nc.vector.tensor_tensor(
                        out=pt[:],
                        in0=dstr_t[:, c:c + 1].to_broadcast([P, P]),
                        in1=iota_t[:],
                        op=mybir.AluOpType.is_equal,
                    )
                    nc.tensor.matmul(out=po[:], lhsT=pt[:],
                                     rhs=mb[:, c - c0, :],
                                     start=(j == 0), stop=(j == kc - 1))
                if ost is None:
                    ost = opool.tile([P, OB, D], f32, tag="ost")
                    o0 = blk
                if stage >= 3:
                    nc.vector.tensor_copy(out=ost[:, blk - o0, :], in_=po[:])
                else:
                    nc.vector.tensor_copy(out=ost[:, blk - o0, :],
                                          in_=mbs[next(iter(mbs))][0][:, 0, :])
                if blk - o0 == OB - 1 or blk == nblocks - 1:
                    nc.sync.dma_start(
                        out=out_r[:, o0:blk + 1, :],
                        in_=ost[:, :blk + 1 - o0, :])
                    ost = None

    nc.compile()
    return nc


def kernel(x, W, b, edge_index, node_rankings):
    x = np.asarray(x, dtype=np.float32)
    W = np.asarray(W, dtype=np.float32)
    b = np.asarray(b, dtype=np.float32)
    edge_index = np.asarray(edge_index)
    node_rankings = np.asarray(node_rankings)

    meta, per_core = _preprocess(x, W, b, edge_index, node_rankings)
    key = (meta["nm_pad"], meta["kc"], meta["nchunks"], meta["nsh_pad"])
    if key not in _cache:
        _cache[key] = _build(meta)
    nc = _cache[key]

    res = run_bass_kernel_spmd(nc, per_core, core_ids=list(range(M)),
                               trace=TRACE)
    LAST["exec_time_ns"] = res.exec_time_ns
    LAST["results"] = res
    outs = [res.results[i]["out"][: meta["nsh"]] for i in range(M)]
    full = np.concatenate(outs, axis=0)[: meta["N"]]
    return full.astype(np.float32)
